# revision 1
# baseline (speedup 1.0000x reference)
"""Trainium2 kernel for nn_MeshTorchLayer_82059645157414.

The reference applies 256 sequential MZI mesh layers to a [4096, 256]
batch of (complexified) states. Every layer is LINEAR in the state, so
the whole mesh (including the gamma phase layer) collapses into one
256x256 complex matrix U with out[b, :] = U @ x[b, :]. Since x is real,
the device-side work is a single real matmul per batch shard:

    out_f32[b, :] = x[b, :] @ W,  W[k, 2j] = Re(U[j,k]), W[k, 2j+1] = Im(U[j,k])

W is composed on host in float64; the [4096,256]x[256,512] matmul runs
data-parallel on 8 NeuronCores (512 rows/core). Device inputs are bf16
(rel err 2.3e-3 vs the fp32 reference, budget 2e-2), output f32.

Per-core schedule (raw Bass, explicit semaphores). The contraction dim
(256 units) splits into k-chunks k0/k1 of 128; the 512 batch rows into
4 tiles t0..t3. Inputs stream as four packed DMAs, sized and queued so
the DMA engines run back-to-back from the first byte and the PE gets
each operand just in time:

  in1 (SP HWDGE #1): [W_k0 | x_k0 t0,t1]       -> a_sb  (data 2.3-2.9us)
  in2 (Pool SWDGE, desc-gen issued at ~1us so its transfer chains with
       no HWDGE/dge gap): [W_k1 | x_k1 t0]     -> b_sb  (data 2.9-3.3us)
  in4 (Act HWDGE, dispatch held behind a DVE memset so SP wins HWDGE
       slot 1): [x_k1 t1 | x_k1 t3]            -> d_sb  (data 3.3-3.5us)
  in3 (SP HWDGE #2): [x_k0 t2,t3 | x_k1 t2]    -> c_sb  (data 3.6-3.9us)

PE: one warmup matmul at ~1.5us (anchors the p-state ramp clock so the
real matmuls run at full clock), then 8 accumulating bf16 matmuls
(213ns each), finishing one tile every ~213-426ns. As each tile's
accumulation closes, DVE and Act copy its PSUM tile to SBUF in
parallel column slices (the only two engines that can read PSUM).
Each tile goes out via its own kv_writeback whose descriptors were
generated on Pool early (prepare_only) and which trigger_dma's straight
onto the DMA engines when that tile's copies land -- the trigger path
skips the per-DMA HWDGE queue (625ns) + dge dispatch (650ns) latency,
and the cost model prices a writeback at batch*d_head/16+1 descriptors.
Output completion rides on the end-of-block engine drains.
"""

import numpy as np
import ml_dtypes

import concourse.bass as bass
import concourse.mybir as mybir
from concourse.bass_utils import run_bass_kernel_spmd

UNITS = 256          # N: state dimension
LAYERS = 256         # L
BATCH = 4096         # B
NCORES = 8
BC = BATCH // NCORES  # 512 batch rows per core
P = 128              # SBUF partitions
NT = BC // P         # 4 batch tiles of 128 rows per core
WF = 2 * UNITS       # 512 interleaved re/im output columns
HB = 2 * P           # 256 batch cols (2 tiles) per a/b input chunk

BF16 = mybir.dt.bfloat16
F32 = mybir.dt.float32

CFG = dict(
    warmup=1,
    in4=True,           # 4th input DMA (x_k1 tile1) on the Act HWDGE queue
    # PSUM->SBUF copy plan: (engine, tile, lo, hi) column slices. Only DVE
    # and Act can read PSUM (GPSIMD cannot); sizes ~inverse to per-column
    # engine cost (DVE 1.04, Act 0.83 ns/col + fixed 125/185ns).
    # HARDWARE CONSTRAINT: a DVE|Act boundary at column >= 252 corrupts the
    # boundary columns on real TRN2 (boundary <= 250 measured safe; the
    # cost model does not know). Keep the split at 250.
    copy_plan=(
        ("dve", 0, 0, 250), ("act", 0, 250, 512),
        ("dve", 1, 0, 250), ("act", 1, 250, 512),
        ("dve", 2, 0, 250), ("act", 2, 250, 512),
        ("dve", 3, 0, 250), ("act", 3, 250, 512),
    ),
)


def _build_w(theta, phi, gamma, mask):
    """Compose the mesh into W [256, 512] f32 (interleaved re/im columns)."""
    theta = np.asarray(theta, np.float64)
    phi = np.asarray(phi, np.float64)
    gamma = np.asarray(gamma, np.float64)
    mask = np.asarray(mask)

    L, M = theta.shape
    N = 2 * M
    m = mask.astype(np.float64)
    th = theta * m + (1 - m) * np.pi
    ph = phi * m + (1 - m) * np.pi
    u = np.exp(1j * th)
    e = np.exp(1j * ph)
    d_top = e * (u - 1) * 0.5
    d_bot = (1 - u) * 0.5
    o_top = 1j * (u + 1) * 0.5
    o_bot = 1j * e * (u + 1) * 0.5
    D = np.stack([d_top, d_bot], axis=-1).reshape(L, N)
    O = np.stack([o_top, o_bot], axis=-1).reshape(L, N)
    odd = (np.arange(L) % 2).astype(bool)
    D[odd] = np.roll(D[odd], 1, axis=1)
    O[odd] = np.roll(O[odd], 1, axis=1)
    base = np.arange(N).reshape(-1, 2)[:, ::-1].reshape(-1)
    oddp = np.concatenate([[0], base[:-2] + 1, [N - 1]])

    U = np.diag(np.exp(1j * gamma)).astype(np.complex128)
    for layer in range(L):
        p = oddp if (layer % 2) else base
        U = D[layer][:, None] * U + O[layer][:, None] * U[p, :]

    W = np.empty((N, 2 * N), np.float32)
    W[:, 0::2] = U.real.T.astype(np.float32)
    W[:, 1::2] = U.imag.T.astype(np.float32)
    return W


def _build_bass(cfg=CFG):
    """Per-core kernel: out[512, 512] f32 = xT.T[512, 256] @ w[256, 512]."""
    nc = bass.Bass()
    use4 = cfg.get("in4", False)
    b_w = WF + P if use4 else WF + HB
    c_w = 3 * P if use4 else 2 * HB
    in1 = nc.dram_tensor("in1", [P, WF + HB], BF16, kind="ExternalInput")
    in2 = nc.dram_tensor("in2", [P, b_w], BF16, kind="ExternalInput")
    in3 = nc.dram_tensor("in3", [P, c_w], BF16, kind="ExternalInput")
    in4 = (nc.dram_tensor("in4", [P, 2 * P], BF16, kind="ExternalInput")
           if use4 else None)
    out = nc.dram_tensor("out", [NT, P, 1, WF], F32, kind="ExternalOutput")

    plan = list(cfg["copy_plan"])
    tile_n = [sum(1 for e in plan if e[1] == t) for t in range(NT)]

    from contextlib import ExitStack

    with ExitStack() as st:
        a_sb = st.enter_context(nc.sbuf_tensor("a_sb", [P, WF + HB], BF16))
        b_sb = st.enter_context(nc.sbuf_tensor("b_sb", [P, b_w], BF16))
        c_sb = st.enter_context(nc.sbuf_tensor("c_sb", [P, c_w], BF16))
        d_sb = (st.enter_context(nc.sbuf_tensor("d_sb", [P, 2 * P], BF16))
                if use4 else None)
        o_sb = st.enter_context(nc.sbuf_tensor("o_sb", [P, 1, NT, WF], F32))
        warm_sb = st.enter_context(nc.sbuf_tensor("warm_sb", [P, P], BF16))
        idx_sb = st.enter_context(nc.sbuf_tensor("idx_sb", [P, NT], mybir.dt.int32))
        acc4 = st.enter_context(nc.psum_tensor("acc4", [P, NT, WF], F32))
        warm_ps = st.enter_context(nc.psum_tensor("warm_ps", [P, P], F32))
        a_sem = st.enter_context(nc.semaphore(name='a_sem'))
        b_sem = st.enter_context(nc.semaphore(name='b_sem'))
        c_sem = st.enter_context(nc.semaphore(name='c_sem'))
        d_sem = st.enter_context(nc.semaphore(name='d_sem'))
        ws_sem = st.enter_context(nc.semaphore(name='ws_sem'))
        idx_sem = st.enter_context(nc.semaphore(name='idx_sem'))
        prep_sem = st.enter_context(nc.semaphore(name='prep_sem'))
        mm_sems = [st.enter_context(nc.semaphore(name=f'mm{t}_sem'))
                   for t in range(NT)]
        cp_sems = [st.enter_context(nc.semaphore(name=f'cp{t}_sem'))
                   for t in range(NT)]
        out_sem = st.enter_context(nc.semaphore(name='out_sem'))
        block = st.enter_context(nc.Block())

        def xk0(t):  # lhsT for k-chunk 0, tile t
            if t < 2:
                return a_sb[:, WF + t * P : WF + (t + 1) * P]
            return c_sb[:, (t - 2) * P : (t - 1) * P]

        def xk1(t):  # lhsT for k-chunk 1, tile t
            if t == 0:
                return b_sb[:, WF : WF + P]
            if t == 1:
                return d_sb[:, :P] if use4 else b_sb[:, WF + P : WF + HB]
            if use4:
                return (c_sb[:, 2 * P : 3 * P] if t == 2
                        else d_sb[:, P : 2 * P])
            return c_sb[:, HB + (t - 2) * P : HB + (t - 1) * P]

        def copy_src(t, lo, hi):
            return acc4[:, t, lo:hi]

        def copy_dst(t, lo, hi):
            return o_sb[:, 0, t, lo:hi]

        @block.sync
        def _(sync):
            sync.dma_start(a_sb[:], in1[:]).then_inc(a_sem, 16)
            sync.dma_start(c_sb[:], in3[:]).then_inc(c_sem, 16)

        @block.gpsimd
        def _(gpsimd):
            # second input chunk via SWDGE: desc-gen runs now (~1us), the
            # transfer chains on the DMA engines right behind input DMA 1
            gpsimd.dma_start(b_sb[:], in2[:]).then_inc(b_sem, 16)
            # kv_writeback ucode lives in the 'attn' GPSIMD library
            from concourse import library_config
            gpsimd.load_library(library_config.attn)
            gpsimd.wait_ge(idx_sem, 1)
            for t in range(NT):
                gpsimd.kv_writeback(
                    out[t : t + 1], o_sb[:, :, t : t + 1, :],
                    idx_sb[:, t : t + 1],
                    prepare_only=True, sem=out_sem,
                ).then_inc(prep_sem, 1)
            for t in range(NT):
                gpsimd.wait_ge(prep_sem, t + 1)
                gpsimd.wait_ge(cp_sems[t], tile_n[t])
                gpsimd.trigger_dma(1)

        @block.vector
        def _(vector):
            if cfg.get("idx_first", True):
                vector.memset(idx_sb[:], 0).then_inc(idx_sem, 1)
                vector.memset(warm_sb[:], 0.0).then_inc(ws_sem, 1)
            else:
                vector.memset(warm_sb[:], 0.0).then_inc(ws_sem, 1)
                vector.memset(idx_sb[:], 0).then_inc(idx_sem, 1)
            for eng, t, lo, hi in plan:
                if eng != "dve":
                    continue
                vector.wait_ge(mm_sems[t], 1)
                vector.tensor_copy(copy_dst(t, lo, hi), copy_src(t, lo, hi)
                                   ).then_inc(cp_sems[t], 1)

        @block.scalar
        def _(scalar):
            if use4:
                # hold this dispatch until DVE's memset lands so SP's first
                # DMA wins the HWDGE queue; Act's slot-2 grant still feeds
                # k1t1 early
                scalar.wait_ge(idx_sem, 1)
                scalar.dma_start(d_sb[:], in4[:]).then_inc(d_sem, 16)
            for eng, t, lo, hi in plan:
                if eng != "act":
                    continue
                scalar.wait_ge(mm_sems[t], 1)
                scalar.copy(copy_dst(t, lo, hi), copy_src(t, lo, hi)
                            ).then_inc(cp_sems[t], 1)

        @block.tensor
        def _(tensor):
            tensor.wait_ge(ws_sem, 1)
            for _ in range(cfg["warmup"]):
                nc.tensor.matmul(
                    warm_ps[:], warm_sb[:], warm_sb[:],
                    start=True, stop=True,
                )
            tensor.wait_ge(a_sem, 16)
            for t in (0, 1):
                nc.tensor.matmul(acc4[:, t, :], xk0(t), a_sb[:, :WF],
                                 start=True, stop=False)
            tensor.wait_ge(b_sem, 16)
            nc.tensor.matmul(acc4[:, 0, :], xk1(0), b_sb[:, :WF],
                             start=False, stop=True).then_inc(mm_sems[0], 1)
            if use4:
                tensor.wait_ge(d_sem, 16)
            nc.tensor.matmul(acc4[:, 1, :], xk1(1), b_sb[:, :WF],
                             start=False, stop=True).then_inc(mm_sems[1], 1)
            tensor.wait_ge(c_sem, 16)
            for t in (2, 3):
                nc.tensor.matmul(acc4[:, t, :], xk0(t), a_sb[:, :WF],
                                 start=True, stop=False)
                nc.tensor.matmul(acc4[:, t, :], xk1(t), b_sb[:, :WF],
                                 start=False, stop=True).then_inc(mm_sems[t], 1)
            # (k1 t3 reads d_sb when use4 -- d_sem was already waited on)

    # populate .instr bytes for extended-inst InstISA subclasses
    # (InstTriggerDma) — raw Bass skips the Bacc pass that does this and
    # the NEFF codegen rejects empty .instr with "ISA wrong length"
    from concourse.library_overlay import lower_extended_insts
    lower_extended_insts(nc)
    return nc


def _pack_inputs(x, W, use4=True):
    """Per-core packed bf16 input chunks (use4 layout):

    in1 [128, 768]: [W_k0 | x_k0 t0 | x_k0 t1]   (SP HWDGE #1)
    in2 [128, 640]: [W_k1 | x_k1 t0]             (Pool SWDGE)
    in4 [128, 256]: [x_k1 t1 | x_k1 t3]          (Act HWDGE)
    in3 [128, 384]: [x_k0 t2 | x_k0 t3 | x_k1 t2] (SP HWDGE #2)
    """
    xT = np.ascontiguousarray(
        x.reshape(NCORES, BC, UNITS).transpose(0, 2, 1)
    ).astype(ml_dtypes.bfloat16)              # [8, 256, 512]
    Wb = W.astype(ml_dtypes.bfloat16)         # [256, 512]
    in1, in2, in3, in4 = [], [], [], []
    for c in range(NCORES):
        in1.append(np.ascontiguousarray(
            np.concatenate([Wb[:P], xT[c, :P, :HB]], axis=1)))
        xk1b = xT[c, P:, :P] if use4 else xT[c, P:, :HB]
        in2.append(np.ascontiguousarray(np.concatenate([Wb[P:], xk1b], axis=1)))
        if use4:
            # in3: [x_k0 t2 | x_k0 t3 | x_k1 t2]; in4: [x_k1 t1 | x_k1 t3]
            in3.append(np.ascontiguousarray(np.concatenate(
                [xT[c, :P, HB:], xT[c, P:, HB:HB + P]], axis=1)))
            in4.append(np.ascontiguousarray(np.concatenate(
                [xT[c, P:, P:HB], xT[c, P:, HB + P:]], axis=1)))
        else:
            in3.append(np.ascontiguousarray(
                np.concatenate([xT[c, :P, HB:], xT[c, P:, HB:]], axis=1)))
            in4.append(np.ascontiguousarray(xT[c, P:, P:HB]))
    return in1, in2, in3, in4


def kernel(x, theta, phi, gamma, mask):
    x = np.ascontiguousarray(np.asarray(x, dtype=np.float32))
    assert x.shape == (BATCH, UNITS)
    W = _build_w(theta, phi, gamma, mask)
    use4 = CFG.get("in4", False)
    in1, in2, in3, in4 = _pack_inputs(x, W, use4)

    nc = _build_bass()
    in_maps = [{"in1": in1[c], "in2": in2[c], "in3": in3[c]}
               for c in range(NCORES)]
    if use4:
        for c in range(NCORES):
            in_maps[c]["in4"] = in4[c]
    res = run_bass_kernel_spmd(nc, in_maps, core_ids=list(range(NCORES)))
    full = np.concatenate(
        [np.ascontiguousarray(np.asarray(r["out"]).reshape(BC, WF))
         for r in res.results],
        axis=0,
    )  # [4096, 512] f32 interleaved re/im
    return full.view(np.complex64)

